# revision 34
# baseline (speedup 1.0000x reference)
"""Bass/Tile kernel for nn_Attention_81690277970645 on TRN2.

Sharding: 8 cores = 2 batches x 4 head-groups (4 heads of d=64 each).
Per core (batch bi, head-group hg):
  inputs:  x_b [2048, 1024], wq/wk/wv slices [1024, 256], bq/bk/bv [256],
           wo slice [256, 1024]
  output:  partial out [2048, 1024] (host sums the 4 head-group partials
           per batch and adds bo)

On-chip dataflow (per core):
  1. x -> x^T via PE transpose (fp32r)     xT   [dim=1024p, seq=2048]
  2. Q^T = wq^T x^T (+bq), K^T likewise    QT,KT [emb=256p, seq=2048] f32r
  3. V  = x wv (no bias; folded later)     V'   [seq=2048p, 4*(64+1)] bf16
     (col 64 of each head's block = 1.0 -> row-sum trick for softmax den)
  4. per i-block (512 seq cols), per head-pair pass, pipelined over j-tiles:
       S^T(jt) = KT_h^T @ QT_h   psum [j=128, 2 heads, i=512] (bf16 in, f32 out)
       expS^T = exp(S/8)         one ScalarE op per pair -> sbuf bf16
       [AV^T | den] += V'_h^T @ expS^T   psum [65, i=512] accum over j (bf16)
       den -> broadcast via fp32r K=1 matmul -> reciprocal_approx_fast (~18 bit)
       outT_h = AV^T * (1/den) + bv_h    sbuf fp32r [64, i]
     (each block's output projection is deferred into the next block's loop
      as PE gap-filler work)
  5. out = outT^T wo (fp32r)              psum [seq=128, 512] -> sbuf -> DMA

Known HW pitfalls hit along the way: DVE reciprocal() on a 1-partition row
costs ~3.3us; reciprocal_approx_fast returns garbage at base partition != 0;
fp32(non-r) matmuls on F32R-bitcast operands corrupt; all matmul-feeding
tensors must be declared float32r end-to-end for the BIR verifier.
"""
import sys
import numpy as np

if '/opt/trn_rl_repo' not in sys.path:
    sys.path.insert(0, '/opt/trn_rl_repo')

import concourse.mybir as mybir
from concourse import bacc
from concourse.tile import TileContext

F32 = mybir.dt.float32
F32R = mybir.dt.float32r
BF16 = mybir.dt.bfloat16

SEQ = 2048
DIM = 1024
EMB_C = 256          # per-core emb columns (4 heads x 64)
NH = 4               # heads per core
DH = 64
SCALE = DH ** -0.5
P = 128
NSEQT = SEQ // P     # 16 seq tiles
NDIMC = DIM // P     # 8 dim chunks
NEMBC = EMB_C // P   # 2 emb chunks
IBLK = 512
NIBLK = SEQ // IBLK  # 4 i-blocks
NJT = SEQ // P       # 16 j tiles


def r(ap):
    return ap.bitcast(F32R)


def build_kernel(row_pack=False):
    nc = bacc.Bacc("TRN2", target_bir_lowering=False, debug=False, num_devices=8)

    x = nc.dram_tensor("x", [SEQ, DIM], F32, kind="ExternalInput")
    wq = nc.dram_tensor("wq", [DIM, EMB_C], F32, kind="ExternalInput")
    wk = nc.dram_tensor("wk", [DIM, EMB_C], F32, kind="ExternalInput")
    wv = nc.dram_tensor("wv", [DIM, EMB_C], F32, kind="ExternalInput")
    bq = nc.dram_tensor("bq", [EMB_C], F32, kind="ExternalInput")
    bk = nc.dram_tensor("bk", [EMB_C], F32, kind="ExternalInput")
    bv = nc.dram_tensor("bv", [EMB_C], F32, kind="ExternalInput")
    wo = nc.dram_tensor("wo", [EMB_C, DIM], F32, kind="ExternalInput")
    ident_d = nc.dram_tensor("ident128", [P, P], F32, kind="ExternalInput")
    ones_d = nc.dram_tensor("ones64", [P, DH], F32, kind="ExternalInput")
    out = nc.dram_tensor("out", [SEQ, DIM], F32, kind="ExternalOutput")

    with TileContext(nc) as tc:
        with (
            tc.tile_pool(name="const", bufs=1) as const_pool,
            tc.tile_pool(name="w", bufs=1) as w_pool,
            tc.tile_pool(name="big", bufs=1) as big_pool,
            tc.tile_pool(name="stage", bufs=3) as stage_pool,
        ):
            # ---- constants / weights ----
            ident = const_pool.tile([P, P], F32R)
            nc.sync.dma_start(ident[:], ident_d[:].bitcast(F32R))
            ones_t = w_pool.tile([P, DH], F32R)
            nc.sync.dma_start(ones_t[:], ones_d[:].bitcast(F32R))

            # ---- stages 1-3: x^T, then Q^T/K^T/V ----
            psA_ctx = tc.tile_pool(name="psA", bufs=1, space="PSUM")
            psA = psA_ctx.__enter__()
            xt_ctx = tc.tile_pool(name="xt", bufs=1)
            xt_pool = xt_ctx.__enter__()
            xT = xt_pool.tile([P, NDIMC, SEQ], F32R)  # [dim_in_chunk, chunk, seq]

            # x^T: 8 seq-tiles per group; batch 8 transposes into a 2-bank psum,
            # one [128,1024] copy per (chunk, group)
            for g in range(NSEQT // 8):
                xss = []
                for si in range(8):
                    s = g * 8 + si
                    xs = xt_pool.tile([P, DIM], F32R, tag="xs", bufs=9, name=f"xs_{s}")
                    nc.sync.dma_start(xs[:], x[s * P:(s + 1) * P, :].bitcast(F32R))
                    xss.append(xs)
                for c in range(NDIMC):
                    pt = psA.tile([P, 8, P], F32R, tag="pt", bufs=2)
                    for si in range(8):
                        nc.tensor.transpose(pt[:, si, :], xss[si][:, c * P:(c + 1) * P], ident[:])
                    # two half-copies so ScalarE and VectorE split the work
                    nc.any.tensor_copy(
                        xT[:, c, g * 8 * P:g * 8 * P + 4 * P], pt[:, :4, :])
                    nc.any.tensor_copy(
                        xT[:, c, g * 8 * P + 4 * P:(g + 1) * 8 * P], pt[:, 4:, :])

            wq_sb = w_pool.tile([P, NDIMC, EMB_C], F32R)
            nc.sync.dma_start(wq_sb[:], wq.rearrange("(c p) e -> p c e", p=P).bitcast(F32R))
            wk_sb = w_pool.tile([P, NDIMC, EMB_C], F32R)
            nc.sync.dma_start(wk_sb[:], wk.rearrange("(c p) e -> p c e", p=P).bitcast(F32R))
            wv_sb = w_pool.tile([P, NDIMC, EMB_C], F32R)
            nc.sync.dma_start(wv_sb[:], wv.rearrange("(c p) e -> p c e", p=P).bitcast(F32R))
            wo_sb = w_pool.tile([P, NEMBC, DIM], F32R)
            nc.sync.dma_start(wo_sb[:], wo.rearrange("(c p) n -> p c n", p=P).bitcast(F32R))
            bq_sb = w_pool.tile([P, NEMBC], F32)
            nc.sync.dma_start(bq_sb[:], bq.rearrange("(c p) -> p c", p=P))
            bk_sb = w_pool.tile([P, NEMBC], F32)
            nc.sync.dma_start(bk_sb[:], bk.rearrange("(c p) -> p c", p=P))
            bv_sb = w_pool.tile([P, NEMBC], F32)
            nc.sync.dma_start(bv_sb[:], bv.rearrange("(c p) -> p c", p=P))

            # Q^T, K^T (+bias via ScalarE)
            QT = big_pool.tile([P, NEMBC, SEQ], F32R)
            KT = big_pool.tile([P, NEMBC, SEQ], F32R)
            for dst, wsb, bsb in ((KT, wk_sb, bk_sb), (QT, wq_sb, bq_sb)):
                for e in range(NEMBC):
                    for ib in range(NIBLK):
                        pq = psA.tile([P, IBLK], F32, tag="pq", bufs=2)
                        for c in range(NDIMC):
                            nc.tensor.matmul(
                                pq[:],
                                wsb[:, c, e * P:(e + 1) * P],
                                xT[:, c, ib * IBLK:(ib + 1) * IBLK],
                                start=(c == 0), stop=(c == NDIMC - 1),
                            )
                        nc.scalar.activation(
                            dst[:, e, ib * IBLK:(ib + 1) * IBLK], pq[:],
                            mybir.ActivationFunctionType.Identity,
                            bias=bsb[:, e:e + 1], scale=1.0,
                        )

            # V' bf16 with ones col per head (bias folded into stage 4)
            VP = big_pool.tile([P, NSEQT, NH * (DH + 1)], BF16)
            for h in range(NH):
                nc.vector.memset(VP[:, :, h * (DH + 1) + DH], 1.0)
            for s in range(NSEQT):
                pv = psA.tile([P, EMB_C], F32, tag="pv", bufs=2)
                for c in range(NDIMC):
                    nc.tensor.matmul(
                        pv[:],
                        xT[:, c, s * P:(s + 1) * P],
                        wv_sb[:, c, :],
                        start=(c == 0), stop=(c == NDIMC - 1),
                    )
                nc.vector.tensor_copy(
                    VP[:, s, :].rearrange("p (h x) -> p h x", h=NH)[:, :, :DH],
                    pv[:].rearrange("p (h d) -> p h d", h=NH),
                )

            xt_ctx.__exit__(None, None, None)
            psA_ctx.__exit__(None, None, None)

            # ---- stages 4+5: attention + output projection, pipelined per jt ----
            psB_ctx = tc.tile_pool(name="psB", bufs=1, space="PSUM")
            psB = psB_ctx.__enter__()
            es_ctx = tc.tile_pool(name="es", bufs=1)
            es_pool = es_ctx.__enter__()

            outT = big_pool.tile([P, NEMBC, SEQ], F32R)

            def emit_spair(ib, jt, hp):
                """S^T for head-pair hp at (ib, jt): one 2-bank psum + one exp."""
                i0 = ib * IBLK
                ps = psB.tile([P, 2, IBLK], F32, tag="s0", bufs=2,
                              name=f"ps{hp}_{ib}_{jt}")
                for hh in range(2):
                    lo = hh * DH
                    nc.tensor.matmul(
                        ps[:, hh, :],
                        KT[lo:lo + DH, hp, jt * P:(jt + 1) * P],
                        QT[lo:lo + DH, hp, i0:i0 + IBLK],
                        start=True, stop=True,
                    )
                es = es_pool.tile([P, 2, IBLK], BF16, tag="es", bufs=4,
                                  name=f"es{hp}_{ib}_{jt}")
                nc.scalar.activation(
                    es[:], ps[:], mybir.ActivationFunctionType.Exp,
                    bias=0.0, scale=SCALE,
                )
                return es

            def emit_av(pavs, es, jt, hp):
                for hh in range(2):
                    h = hp * 2 + hh
                    nc.tensor.matmul(
                        pavs[hh][:DH + 1, :],
                        VP[:, jt, h * (DH + 1):(h + 1) * (DH + 1)],
                        es[:, hh, :],
                        start=(jt == 0), stop=(jt == NJT - 1),
                    )

            def oproj_units(ib):
                units = []
                for s in range(ib * (IBLK // P), (ib + 1) * (IBLK // P)):
                    for nb in range(DIM // IBLK):
                        def go(s=s, nb=nb):
                            po = psB.tile([P, IBLK], F32, tag="po", bufs=2,
                                          name=f"po_{s}_{nb}")
                            for e in range(NEMBC):
                                nc.tensor.matmul(
                                    po[:],
                                    outT[:, e, s * P:(s + 1) * P],
                                    wo_sb[:, e, nb * IBLK:(nb + 1) * IBLK],
                                    start=(e == 0), stop=(e == NEMBC - 1),
                                )
                            oc = stage_pool.tile([P, IBLK], F32, tag="oc", bufs=2)
                            nc.any.tensor_copy(oc[:], po[:])
                            nc.sync.dma_start(
                                out[s * P:(s + 1) * P, nb * IBLK:(nb + 1) * IBLK], oc[:]
                            )
                        units.append(go)
                return units

            pending = []
            div2 = []
            for ib in range(NIBLK):
                i0 = ib * IBLK
                for hp in range(2):
                    pavs = [
                        psB.tile([P, IBLK], F32, tag="pav", bufs=2,
                                 name=f"pav_{hp}_{hh}_{ib}")
                        for hh in range(2)
                    ]
                    prev = None
                    n_fill = len(pending)
                    for jt in range(NJT):
                        es = emit_spair(ib, jt, hp)
                        # previous pass's divide tail, behind fresh S work so
                        # its DVE inputs have drained by the time PE reaches it
                        if div2 and jt < 2:
                            div2.pop(0)()
                        if prev is not None:
                            emit_av(pavs, prev, jt - 1, hp)
                        if n_fill > 0 and jt % 2 == 1:
                            pending.pop(0)()
                            n_fill -= 1
                        prev = es
                    emit_av(pavs, prev, NJT - 1, hp)

                    # copy AV accumulators to SBUF (releases psum slots fast);
                    # the divide's matmul half is deferred into the next pass
                    for hh in range(2):
                        h = hp * 2 + hh
                        pavc = stage_pool.tile([DH, IBLK], F32, tag="pavc", bufs=2,
                                               name=f"pavc_{h}_{ib}")
                        nc.vector.tensor_copy(pavc[:], pavs[hh][:DH, :])
                        den_row = stage_pool.tile([1, IBLK], F32R, tag="den_row",
                                                  bufs=2, name=f"den_{h}_{ib}")
                        nc.vector.tensor_copy(
                            den_row[:], pavs[hh][DH:DH + 1, :].bitcast(F32R))

                        # broadcast den across 64 partitions (fp32r K=1 matmul),
                        # then ~18-bit reciprocal in one custom-DVE op at base 0
                        # (plain reciprocal() on a 1-partition row costs ~3.3us;
                        # reciprocal_approx_fast at base!=0 returns garbage)
                        def div_tail(h=h, i0=i0, ib=ib, pavc=pavc, den_row=den_row):
                            recb_ps = psB.tile([P, IBLK], F32, tag="po", bufs=2,
                                               name=f"recb_{h}_{ib}")
                            nc.tensor.matmul(
                                recb_ps[:DH, :], ones_t[0:1, :], den_row[:],
                                start=True, stop=True,
                            )
                            recb_sb = stage_pool.tile([DH, IBLK], F32, tag="recb", bufs=2)
                            nc.vector.reciprocal_approx_fast(recb_sb[:], recb_ps[:DH, :])
                            e_c, e_lo = divmod(h * DH, P)
                            dst = outT[e_lo:e_lo + DH, e_c, i0:i0 + IBLK]
                            nc.vector.tensor_tensor(
                                dst, pavc[:], recb_sb[:],
                                mybir.AluOpType.mult,
                            )
                            nc.vector.tensor_scalar_add(
                                dst, dst, bv_sb[e_lo:e_lo + DH, e_c:e_c + 1])
                        div2.append(div_tail)

                # output projection deferred into the next block's S loop
                pending.extend(oproj_units(ib))

            for go in div2:
                go()
            for go in pending:
                go()

            es_ctx.__exit__(None, None, None)
            psB_ctx.__exit__(None, None, None)

    nc.compile()
    return nc


def shard_inputs(inputs):
    """Full inputs dict -> list of 8 per-core input dicts."""
    x = np.ascontiguousarray(inputs["x"], dtype=np.float32)
    maps = []
    for core in range(8):
        bi, hg = divmod(core, 4)
        sl = slice(hg * EMB_C, (hg + 1) * EMB_C)
        maps.append({
            "x": np.ascontiguousarray(x[bi]),
            "wq": np.ascontiguousarray(inputs["wq"][:, sl], np.float32),
            "wk": np.ascontiguousarray(inputs["wk"][:, sl], np.float32),
            "wv": np.ascontiguousarray(inputs["wv"][:, sl], np.float32),
            "bq": np.ascontiguousarray(inputs["bq"][sl], np.float32),
            "bk": np.ascontiguousarray(inputs["bk"][sl], np.float32),
            "bv": np.ascontiguousarray(inputs["bv"][sl], np.float32),
            "wo": np.ascontiguousarray(inputs["wo"][sl, :], np.float32),
            "ident128": np.eye(P, dtype=np.float32),
            "ones64": np.ones((P, DH), np.float32),
        })
    return maps


def gather_outputs(results, bo):
    out = np.zeros((2, SEQ, DIM), np.float32)
    for core in range(8):
        bi = core // 4
        out[bi] += results[core]["out"]
    out += bo.astype(np.float32)
    return out


_NC_CACHE = {}


def _get_nc(row_pack=True):
    if row_pack not in _NC_CACHE:
        _NC_CACHE[row_pack] = build_kernel(row_pack=row_pack)
    return _NC_CACHE[row_pack]


def run_sharded(inputs, trace=False, row_pack=True):
    """Returns (full_output [2,2048,1024] fp32, BassKernelResults)."""
    from concourse import bass_utils
    nc = _get_nc(row_pack)
    maps = shard_inputs(inputs)
    res = bass_utils.run_bass_kernel_spmd(
        nc, maps, core_ids=list(range(8)), trace=trace,
    )
    out = gather_outputs(res.results, np.asarray(inputs["bo"]))
    return out, res


def kernel(**inputs):
    out, _ = run_sharded(inputs)
    return out


# revision 35
# speedup vs baseline: 1.0821x; 1.0821x over previous
"""Bass/Tile kernel for nn_Attention_81690277970645 on TRN2.

Sharding: 8 cores = 2 batches x 4 head-groups (4 heads of d=64 each).
Per core (batch bi, head-group hg):
  inputs:  x_b [2048, 1024], wq/wk/wv slices [1024, 256], bq/bk/bv [256],
           wo slice [256, 1024]
  output:  partial out [2048, 1024] (host sums the 4 head-group partials
           per batch and adds bo)

On-chip dataflow (per core):
  1. x -> x^T via PE transpose (fp32r)     xT   [dim=1024p, seq=2048]
  2. Q^T = wq^T x^T (+bq), K^T likewise    QT,KT [emb=256p, seq=2048] bf16
  3. V  = x wv (no bias; folded later)     V'   [seq=2048p, 4*(64+1)] bf16
     (col 64 of each head's block = 1.0 -> row-sum trick for softmax den)
  4. per i-block (512 seq cols), per head-pair pass, pipelined over j-tiles:
       S^T(jt) = KT_h^T @ QT_h   psum [j=128, 2 heads, i=512] (bf16 in, f32 out)
       expS^T = exp(S/8)         one ScalarE op per pair -> sbuf bf16
       [AV^T | den] += V'_h^T @ expS^T   psum [65, i=512] accum over j (bf16)
       den -> broadcast via fp32r K=1 matmul -> reciprocal_approx_fast (~18 bit)
       outT_h = AV^T * (1/den) + bv_h    sbuf fp32r [64, i]
     (each block's output projection is deferred into the next block's loop
      as PE gap-filler work)
  5. out = outT^T wo (fp32r)              psum [seq=128, 512] -> sbuf -> DMA

Known HW pitfalls hit along the way: DVE reciprocal() on a 1-partition row
costs ~3.3us; reciprocal_approx_fast returns garbage at base partition != 0;
fp32(non-r) matmuls on F32R-bitcast operands corrupt; all matmul-feeding
tensors must be declared float32r end-to-end for the BIR verifier.
"""
import sys
import numpy as np

if '/opt/trn_rl_repo' not in sys.path:
    sys.path.insert(0, '/opt/trn_rl_repo')

import concourse.mybir as mybir
from concourse import bacc
from concourse.tile import TileContext

F32 = mybir.dt.float32
F32R = mybir.dt.float32r
BF16 = mybir.dt.bfloat16

SEQ = 2048
DIM = 1024
EMB_C = 256          # per-core emb columns (4 heads x 64)
NH = 4               # heads per core
DH = 64
SCALE = DH ** -0.5
P = 128
NSEQT = SEQ // P     # 16 seq tiles
NDIMC = DIM // P     # 8 dim chunks
NEMBC = EMB_C // P   # 2 emb chunks
IBLK = 512
NIBLK = SEQ // IBLK  # 4 i-blocks
NJT = SEQ // P       # 16 j tiles


def r(ap):
    return ap.bitcast(F32R)


def build_kernel(row_pack=False):
    nc = bacc.Bacc("TRN2", target_bir_lowering=False, debug=False, num_devices=8)

    x = nc.dram_tensor("x", [SEQ, DIM], F32, kind="ExternalInput")
    wq = nc.dram_tensor("wq", [DIM, EMB_C], F32, kind="ExternalInput")
    wk = nc.dram_tensor("wk", [DIM, EMB_C], F32, kind="ExternalInput")
    wv = nc.dram_tensor("wv", [DIM, EMB_C], F32, kind="ExternalInput")
    bq = nc.dram_tensor("bq", [EMB_C], F32, kind="ExternalInput")
    bk = nc.dram_tensor("bk", [EMB_C], F32, kind="ExternalInput")
    bv = nc.dram_tensor("bv", [EMB_C], F32, kind="ExternalInput")
    wo = nc.dram_tensor("wo", [EMB_C, DIM], F32, kind="ExternalInput")
    ident_d = nc.dram_tensor("ident128", [P, P], F32, kind="ExternalInput")
    ones_d = nc.dram_tensor("ones64", [P, DH], F32, kind="ExternalInput")
    out = nc.dram_tensor("out", [SEQ, DIM], F32, kind="ExternalOutput")

    with TileContext(nc) as tc:
        with (
            tc.tile_pool(name="const", bufs=1) as const_pool,
            tc.tile_pool(name="w", bufs=1) as w_pool,
            tc.tile_pool(name="big", bufs=1) as big_pool,
            tc.tile_pool(name="stage", bufs=3) as stage_pool,
        ):
            # ---- constants / weights ----
            ident = const_pool.tile([P, P], F32R)
            nc.sync.dma_start(ident[:], ident_d[:].bitcast(F32R))
            ones_t = w_pool.tile([P, DH], F32R)
            nc.sync.dma_start(ones_t[:], ones_d[:].bitcast(F32R))

            # ---- stages 1-3: x^T, then Q^T/K^T/V ----
            psA_ctx = tc.tile_pool(name="psA", bufs=1, space="PSUM")
            psA = psA_ctx.__enter__()
            xt_ctx = tc.tile_pool(name="xt", bufs=1)
            xt_pool = xt_ctx.__enter__()
            xT = xt_pool.tile([P, NDIMC, SEQ], F32R)  # [dim_in_chunk, chunk, seq]

            # x^T: 8 seq-tiles per group; batch 8 transposes into a 2-bank psum,
            # one [128,1024] copy per (chunk, group)
            for g in range(NSEQT // 8):
                xss = []
                for si in range(8):
                    s = g * 8 + si
                    xs = xt_pool.tile([P, DIM], F32R, tag="xs", bufs=9, name=f"xs_{s}")
                    nc.sync.dma_start(xs[:], x[s * P:(s + 1) * P, :].bitcast(F32R))
                    xss.append(xs)
                for c in range(NDIMC):
                    pt = psA.tile([P, 8, P], F32R, tag="pt", bufs=2)
                    for si in range(8):
                        nc.tensor.transpose(pt[:, si, :], xss[si][:, c * P:(c + 1) * P], ident[:])
                    # two half-copies so ScalarE and VectorE split the work
                    nc.any.tensor_copy(
                        xT[:, c, g * 8 * P:g * 8 * P + 4 * P], pt[:, :4, :])
                    nc.any.tensor_copy(
                        xT[:, c, g * 8 * P + 4 * P:(g + 1) * 8 * P], pt[:, 4:, :])

            wq_sb = w_pool.tile([P, NDIMC, EMB_C], F32R)
            nc.sync.dma_start(wq_sb[:], wq.rearrange("(c p) e -> p c e", p=P).bitcast(F32R))
            wk_sb = w_pool.tile([P, NDIMC, EMB_C], F32R)
            nc.sync.dma_start(wk_sb[:], wk.rearrange("(c p) e -> p c e", p=P).bitcast(F32R))
            wv_sb = w_pool.tile([P, NDIMC, EMB_C], F32R)
            nc.sync.dma_start(wv_sb[:], wv.rearrange("(c p) e -> p c e", p=P).bitcast(F32R))
            wo_sb = w_pool.tile([P, NEMBC, DIM], F32R)
            nc.sync.dma_start(wo_sb[:], wo.rearrange("(c p) n -> p c n", p=P).bitcast(F32R))
            bq_sb = w_pool.tile([P, NEMBC], F32)
            nc.sync.dma_start(bq_sb[:], bq.rearrange("(c p) -> p c", p=P))
            bk_sb = w_pool.tile([P, NEMBC], F32)
            nc.sync.dma_start(bk_sb[:], bk.rearrange("(c p) -> p c", p=P))
            bv_sb = w_pool.tile([P, NEMBC], F32)
            nc.sync.dma_start(bv_sb[:], bv.rearrange("(c p) -> p c", p=P))

            # Q^T, K^T (+bias via ScalarE)
            QT = big_pool.tile([P, NEMBC, SEQ], BF16)
            KT = big_pool.tile([P, NEMBC, SEQ], BF16)
            for dst, wsb, bsb in ((KT, wk_sb, bk_sb), (QT, wq_sb, bq_sb)):
                for e in range(NEMBC):
                    for ib in range(NIBLK):
                        pq = psA.tile([P, IBLK], F32, tag="pq", bufs=2)
                        for c in range(NDIMC):
                            nc.tensor.matmul(
                                pq[:],
                                wsb[:, c, e * P:(e + 1) * P],
                                xT[:, c, ib * IBLK:(ib + 1) * IBLK],
                                start=(c == 0), stop=(c == NDIMC - 1),
                            )
                        nc.scalar.activation(
                            dst[:, e, ib * IBLK:(ib + 1) * IBLK], pq[:],
                            mybir.ActivationFunctionType.Identity,
                            bias=bsb[:, e:e + 1], scale=1.0,
                        )

            # V' bf16 with ones col per head (bias folded into stage 4)
            VP = big_pool.tile([P, NSEQT, NH * (DH + 1)], BF16)
            for h in range(NH):
                nc.vector.memset(VP[:, :, h * (DH + 1) + DH], 1.0)
            for s in range(NSEQT):
                pv = psA.tile([P, EMB_C], F32, tag="pv", bufs=2)
                for c in range(NDIMC):
                    nc.tensor.matmul(
                        pv[:],
                        xT[:, c, s * P:(s + 1) * P],
                        wv_sb[:, c, :],
                        start=(c == 0), stop=(c == NDIMC - 1),
                    )
                nc.vector.tensor_copy(
                    VP[:, s, :].rearrange("p (h x) -> p h x", h=NH)[:, :, :DH],
                    pv[:].rearrange("p (h d) -> p h d", h=NH),
                )

            xt_ctx.__exit__(None, None, None)
            psA_ctx.__exit__(None, None, None)

            # ---- stages 4+5: attention + output projection, pipelined per jt ----
            psB_ctx = tc.tile_pool(name="psB", bufs=1, space="PSUM")
            psB = psB_ctx.__enter__()
            es_ctx = tc.tile_pool(name="es", bufs=1)
            es_pool = es_ctx.__enter__()

            outT = big_pool.tile([P, NEMBC, SEQ], F32R)

            def emit_spair(ib, jt, hp):
                """S^T for head-pair hp at (ib, jt): one 2-bank psum + one exp."""
                i0 = ib * IBLK
                ps = psB.tile([P, 2, IBLK], F32, tag="s0", bufs=2,
                              name=f"ps{hp}_{ib}_{jt}")
                for hh in range(2):
                    lo = hh * DH
                    nc.tensor.matmul(
                        ps[:, hh, :],
                        KT[lo:lo + DH, hp, jt * P:(jt + 1) * P],
                        QT[lo:lo + DH, hp, i0:i0 + IBLK],
                        start=True, stop=True,
                    )
                es = es_pool.tile([P, 2, IBLK], BF16, tag="es", bufs=4,
                                  name=f"es{hp}_{ib}_{jt}")
                nc.scalar.activation(
                    es[:], ps[:], mybir.ActivationFunctionType.Exp,
                    bias=0.0, scale=SCALE,
                )
                return es

            def emit_av(pavs, es, jt, hp):
                for hh in range(2):
                    h = hp * 2 + hh
                    nc.tensor.matmul(
                        pavs[hh][:DH + 1, :],
                        VP[:, jt, h * (DH + 1):(h + 1) * (DH + 1)],
                        es[:, hh, :],
                        start=(jt == 0), stop=(jt == NJT - 1),
                    )

            def oproj_units(ib):
                units = []
                for s in range(ib * (IBLK // P), (ib + 1) * (IBLK // P)):
                    for nb in range(DIM // IBLK):
                        def go(s=s, nb=nb):
                            po = psB.tile([P, IBLK], F32, tag="po", bufs=2,
                                          name=f"po_{s}_{nb}")
                            for e in range(NEMBC):
                                nc.tensor.matmul(
                                    po[:],
                                    outT[:, e, s * P:(s + 1) * P],
                                    wo_sb[:, e, nb * IBLK:(nb + 1) * IBLK],
                                    start=(e == 0), stop=(e == NEMBC - 1),
                                )
                            oc = stage_pool.tile([P, IBLK], F32, tag="oc", bufs=2)
                            nc.any.tensor_copy(oc[:], po[:])
                            nc.sync.dma_start(
                                out[s * P:(s + 1) * P, nb * IBLK:(nb + 1) * IBLK], oc[:]
                            )
                        units.append(go)
                return units

            pending = []
            div2 = []
            for ib in range(NIBLK):
                i0 = ib * IBLK
                for hp in range(2):
                    pavs = [
                        psB.tile([P, IBLK], F32, tag="pav", bufs=2,
                                 name=f"pav_{hp}_{hh}_{ib}")
                        for hh in range(2)
                    ]
                    prev = None
                    n_fill = len(pending)
                    for jt in range(NJT):
                        es = emit_spair(ib, jt, hp)
                        # previous pass's divide tail, behind fresh S work so
                        # its DVE inputs have drained by the time PE reaches it
                        if div2 and jt < 2:
                            div2.pop(0)()
                        if prev is not None:
                            emit_av(pavs, prev, jt - 1, hp)
                        if n_fill > 0 and jt % 2 == 1:
                            pending.pop(0)()
                            n_fill -= 1
                        prev = es
                    emit_av(pavs, prev, NJT - 1, hp)

                    # copy AV accumulators to SBUF (releases psum slots fast);
                    # the divide's matmul half is deferred into the next pass
                    for hh in range(2):
                        h = hp * 2 + hh
                        pavc = stage_pool.tile([DH, IBLK], F32, tag="pavc", bufs=2,
                                               name=f"pavc_{h}_{ib}")
                        nc.vector.tensor_copy(pavc[:], pavs[hh][:DH, :])
                        den_row = stage_pool.tile([1, IBLK], F32R, tag="den_row",
                                                  bufs=2, name=f"den_{h}_{ib}")
                        nc.vector.tensor_copy(
                            den_row[:], pavs[hh][DH:DH + 1, :].bitcast(F32R))

                        # broadcast den across 64 partitions (fp32r K=1 matmul),
                        # then ~18-bit reciprocal in one custom-DVE op at base 0
                        # (plain reciprocal() on a 1-partition row costs ~3.3us;
                        # reciprocal_approx_fast at base!=0 returns garbage)
                        def div_tail(h=h, i0=i0, ib=ib, pavc=pavc, den_row=den_row):
                            recb_ps = psB.tile([P, IBLK], F32, tag="po", bufs=2,
                                               name=f"recb_{h}_{ib}")
                            nc.tensor.matmul(
                                recb_ps[:DH, :], ones_t[0:1, :], den_row[:],
                                start=True, stop=True,
                            )
                            recb_sb = stage_pool.tile([DH, IBLK], F32, tag="recb", bufs=2)
                            nc.vector.reciprocal_approx_fast(recb_sb[:], recb_ps[:DH, :])
                            e_c, e_lo = divmod(h * DH, P)
                            dst = outT[e_lo:e_lo + DH, e_c, i0:i0 + IBLK]
                            nc.vector.tensor_tensor(
                                dst, pavc[:], recb_sb[:],
                                mybir.AluOpType.mult,
                            )
                            nc.vector.tensor_scalar_add(
                                dst, dst, bv_sb[e_lo:e_lo + DH, e_c:e_c + 1])
                        div2.append(div_tail)

                # output projection deferred into the next block's S loop
                pending.extend(oproj_units(ib))

            for go in div2:
                go()
            for go in pending:
                go()

            es_ctx.__exit__(None, None, None)
            psB_ctx.__exit__(None, None, None)

    nc.compile()
    return nc


def shard_inputs(inputs):
    """Full inputs dict -> list of 8 per-core input dicts."""
    x = np.ascontiguousarray(inputs["x"], dtype=np.float32)
    maps = []
    for core in range(8):
        bi, hg = divmod(core, 4)
        sl = slice(hg * EMB_C, (hg + 1) * EMB_C)
        maps.append({
            "x": np.ascontiguousarray(x[bi]),
            "wq": np.ascontiguousarray(inputs["wq"][:, sl], np.float32),
            "wk": np.ascontiguousarray(inputs["wk"][:, sl], np.float32),
            "wv": np.ascontiguousarray(inputs["wv"][:, sl], np.float32),
            "bq": np.ascontiguousarray(inputs["bq"][sl], np.float32),
            "bk": np.ascontiguousarray(inputs["bk"][sl], np.float32),
            "bv": np.ascontiguousarray(inputs["bv"][sl], np.float32),
            "wo": np.ascontiguousarray(inputs["wo"][sl, :], np.float32),
            "ident128": np.eye(P, dtype=np.float32),
            "ones64": np.ones((P, DH), np.float32),
        })
    return maps


def gather_outputs(results, bo):
    out = np.zeros((2, SEQ, DIM), np.float32)
    for core in range(8):
        bi = core // 4
        out[bi] += results[core]["out"]
    out += bo.astype(np.float32)
    return out


_NC_CACHE = {}


def _get_nc(row_pack=True):
    if row_pack not in _NC_CACHE:
        _NC_CACHE[row_pack] = build_kernel(row_pack=row_pack)
    return _NC_CACHE[row_pack]


def run_sharded(inputs, trace=False, row_pack=True):
    """Returns (full_output [2,2048,1024] fp32, BassKernelResults)."""
    from concourse import bass_utils
    nc = _get_nc(row_pack)
    maps = shard_inputs(inputs)
    res = bass_utils.run_bass_kernel_spmd(
        nc, maps, core_ids=list(range(8)), trace=trace,
    )
    out = gather_outputs(res.results, np.asarray(inputs["bo"]))
    return out, res


def kernel(**inputs):
    out, _ = run_sharded(inputs)
    return out
